# revision 7
# baseline (speedup 1.0000x reference)
"""NavierStokesSplittingEuler trn2 kernel, 8-core SPMD.

Sharding: x-axis 4-way per channel (core c: channel c//4, x-rows
[128*(c%4), 128*(c%4)+128)).  The 1000 Jacobi iterations run as 20
applications of the fused 50-step operator  phi <- S^50 phi - B50
(S = quarter neighbor-sum; B50 = sum_{j<50} S^j B' precomputed on
device).  Each application is a stack of PSUM-accumulated PE matmuls:
for each y-offset dy, the x-coupling is a banded stationary matrix and
the y-shift is a strided read of a wrap-padded SBUF buffer.  The 50-row
halo is refreshed once per phase with a per-channel AllGather; each
core selects its ring neighbors' rows with partition-id-driven dynamic
DMA offsets.
"""
import os
import sys

for _p in ("/opt/trn_rl_repo", "/root/.axon_site/_ro/trn_rl_repo"):
    if os.path.isdir(_p) and _p not in sys.path:
        sys.path.append(_p)

import numpy as np
import concourse.bass as bass
import concourse.tile as tile
from concourse import bacc, mybir
from concourse.bass import ds
from concourse.bass_utils import run_bass_kernel_spmd

F32 = mybir.dt.float32
N = 512
C = 2
NCORE = 8
K = 50           # fused steps per phase
NPH = int(os.environ.get("NSK_NPH", "20"))  # phases; K*NPH iterations
OWN = 128        # owned x-rows per core
H2 = 2 * K       # halo rows (50 up + 50 down)
TB = 122         # V_star work-tile rows (2 tiles, overlapping)
PADL = 52
PADR = 60
BOFF = 118       # B-tile row offset in the input block
BLK = OWN + PADL + PADR  # 240 input rows per core
W = N + 2 * K            # wrap-padded width

DT, BETA, RHO, NU = 0.1, 0.5, 1.0, 0.1
CADV = -DT
CLAPC = 1.0 - 4.0 * DT * NU
CLAPN = DT * NU
CGP = -DT * BETA / RHO / 2.0    # coeff on raw (P[+1]-P[-1]) diffs
CBD = RHO / (4.0 * DT) / 2.0    # B' = CBD*(xdiff+ydiff) = 1.25*...
CGF = -DT / RHO / 2.0           # V_new grad(Phi) coeff on raw diffs


def _taps(k):
    t = np.array([[1.0]])
    for _ in range(k):
        p = np.zeros((t.shape[0] + 2, t.shape[1] + 2))
        for dx, dy in ((-1, 0), (1, 0), (0, -1), (0, 1)):
            p[1 + dx:1 + dx + t.shape[0], 1 + dy:1 + dy + t.shape[1]] += 0.25 * t
        t = p
    return t


def _qtaps(k):
    acc = np.zeros((2 * k - 1, 2 * k - 1))
    cur = np.array([[1.0]])
    for j in range(k):
        r = (2 * k - 1 - cur.shape[0]) // 2
        acc[r:r + cur.shape[0], r:r + cur.shape[1]] += cur
        if j < k - 1:
            p = np.zeros((cur.shape[0] + 2, cur.shape[1] + 2))
            for dx, dy in ((-1, 0), (1, 0), (0, -1), (0, 1)):
                p[1 + dx:1 + dx + cur.shape[0], 1 + dy:1 + dy + cur.shape[1]] += 0.25 * cur
            cur = p
    return acc


def _band(nrows, ncols, entries):
    s = np.zeros((nrows, ncols), np.float32)
    for off, cf in entries.items():
        for m in range(ncols):
            kk = m + off
            if 0 <= kk < nrows:
                s[kk, m] += cf
    return s


def _stack_t(stack):
    """[nq, P, M] -> host layout [P, nq*M] so DMA is a plain 2D copy."""
    nq, p, m = stack.shape
    return np.ascontiguousarray(
        np.transpose(stack, (1, 0, 2)).reshape(p, nq * m).astype(np.float32))


def _build_consts():
    tp = _taps(K)
    qt = _qtaps(K)

    def tap(dx, dy):
        return tp[dx + K, dy + K] if abs(dx) <= K and abs(dy) <= K else 0.0

    def qtap(dx, dy):
        r = K - 1
        return qt[dx + r, dy + r] if abs(dx) <= r and abs(dy) <= r else 0.0

    mst = np.zeros((2 * K + 1, OWN, OWN), np.float64)
    for dyi, dy in enumerate(range(-K, K + 1)):
        for d in range(-K, K + 1):
            w = tap(d, dy)
            if w:
                for m in range(max(0, -d), min(OWN, OWN - d)):
                    mst[dyi, m + d, m] = w
    hst = np.zeros((2 * K - 1, H2, OWN), np.float64)
    for dyi, dy in enumerate(range(-(K - 1), K)):
        for r in range(H2):
            g = (r - K) if r < K else (OWN + r - K)
            for m in range(OWN):
                w = tap(g - m, dy)
                if w:
                    hst[dyi, r, m] = w
    mqt = np.zeros((2 * K - 1, OWN, OWN), np.float64)
    for dyi, dy in enumerate(range(-(K - 1), K)):
        for d in range(-(K - 1), K):
            w = qtap(d, dy)
            if w:
                for m in range(max(0, -d), min(OWN, OWN - d)):
                    mqt[dyi, m + d, m] = w
    hqt = np.zeros((2 * K - 3, H2, OWN), np.float64)
    for dyi, dy in enumerate(range(-(K - 2), K - 1)):
        for r in range(H2):
            g = (r - K) if r < K else (OWN + r - K)
            for m in range(OWN):
                w = qtap(g - m, dy)
                if w:
                    hqt[dyi, r, m] = w

    sml = {}
    sml["lin"] = _band(TB, TB, {0: CLAPC, 1: CLAPN, -1: CLAPN})
    sml["eyelapn"] = (CLAPN * np.eye(TB)).astype(np.float32)
    sml["gx"] = _band(TB, TB, {1: 0.5, -1: -0.5})
    sml["gpx"] = _band(TB, TB, {1: CGP, -1: -CGP})
    sml["eyegp"] = (CGP * np.eye(TB)).astype(np.float32)
    sml["eyegpn"] = (-CGP * np.eye(TB)).astype(np.float32)
    sml["bdx"] = _band(TB, TB, {1: CBD, -1: -CBD})
    sml["negi"] = (-np.eye(OWN)).astype(np.float32)
    sml["gphi"] = _band(OWN, OWN, {1: CGF, -1: -CGF})
    gph = np.zeros((H2, OWN), np.float32)
    gph[K - 1, 0] = -CGF
    gph[K, OWN - 1] = CGF
    sml["gphih"] = gph
    sela = np.zeros((TB, OWN), np.float32)
    for m in range(0, 69):
        sela[m + 52, m] = 1.0
    selb = np.zeros((TB, OWN), np.float32)
    for m in range(69, OWN):
        selb[m - 66, m] = 1.0
    sml["sela"] = sela
    sml["selb"] = selb

    consts = {"mst": _stack_t(mst), "hst": _stack_t(hst),
              "mqt": _stack_t(mqt), "hqt": _stack_t(hqt)}
    consts.update(sml)
    return consts


_PROG = None


def _build_program(consts):
    nc = bacc.Bacc("TRN2", target_bir_lowering=False, debug=False,
                   enable_asserts=True, num_devices=NCORE)
    vblk = nc.declare_dram_parameter("vblk", [2, BLK, N], F32, isOutput=False)
    pblk = nc.declare_dram_parameter("pblk", [BLK, N], F32, isOutput=False)
    dram_in = {k: nc.declare_dram_parameter(k, list(v.shape), F32, isOutput=False)
               for k, v in consts.items()}
    vout = nc.declare_dram_parameter("vout", [2, OWN, N], F32, isOutput=True)
    pout = nc.declare_dram_parameter("pout", [OWN, N], F32, isOutput=True)

    groups = [[0, 1, 2, 3], [4, 5, 6, 7]]
    AOP = mybir.AluOpType
    SMALL = ("lin", "eyelapn", "gx", "gpx", "eyegp", "eyegpn", "bdx",
             "negi", "gphi", "gphih", "sela", "selb")

    with tile.TileContext(nc) as tc:
        with tc.tile_pool(name="const", bufs=1) as cpool, \
             tc.tile_pool(name="data", bufs=1) as dpool, \
             tc.tile_pool(name="phib", bufs=2) as phipool, \
             tc.tile_pool(name="qstream", bufs=2) as qpool, \
             tc.tile_pool(name="scratch", bufs=1) as spool, \
             tc.tile_pool(name="psph", bufs=2, space="PSUM") as pspool, \
             tc.tile_pool(name="psaux", bufs=2, space="PSUM") as pspool2, \
             tc.tile_pool(name="dram", bufs=2, space="DRAM") as drpool:

            # resident stationaries (plain 2D copies thanks to host layout)
            mst = cpool.tile([OWN, (2 * K + 1) * OWN], F32, tag="mst")
            nc.sync.dma_start(mst[:], dram_in["mst"][:])
            hst = cpool.tile([H2, (2 * K - 1) * OWN], F32, tag="hst")
            nc.sync.dma_start(hst[:], dram_in["hst"][:])
            smt = {}
            for k in SMALL:
                smt[k] = cpool.tile(list(consts[k].shape), F32, tag=f"sm_{k}",
                                    name=f"sm_{k}")
                nc.sync.dma_start(smt[k][:], dram_in[k][:])

            # input field tiles: A = blk rows 0..117, B = rows 114..231
            va = dpool.tile([TB, N + 2], F32, tag="va0")
            vb = dpool.tile([TB, N + 2], F32, tag="vb0")
            wa = dpool.tile([TB, N + 2], F32, tag="wa1")
            wb = dpool.tile([TB, N + 2], F32, tag="wb1")
            pa = dpool.tile([TB, N + 2], F32, tag="pa")
            pb = dpool.tile([TB, N + 2], F32, tag="pb")
            for t, src, r0 in ((va, 0, 0), (vb, 0, BOFF), (wa, 1, 0), (wb, 1, BOFF)):
                nc.sync.dma_start(t[:, 1:N + 1], vblk[src, r0:r0 + TB, :])
                nc.sync.dma_start(t[:, 0:1], vblk[src, r0:r0 + TB, N - 1:N])
                nc.sync.dma_start(t[:, N + 1:N + 2], vblk[src, r0:r0 + TB, 0:1])
            for t, r0 in ((pa, 0), (pb, BOFF)):
                nc.sync.dma_start(t[:, 1:N + 1], pblk[r0:r0 + TB, :])
                nc.sync.dma_start(t[:, 0:1], pblk[r0:r0 + TB, N - 1:N])
                nc.sync.dma_start(t[:, N + 1:N + 2], pblk[r0:r0 + TB, 0:1])
            pown = dpool.tile([OWN, N], F32, tag="pown")
            nc.sync.dma_start(pown[:], pblk[PADL:PADL + OWN, :])

            # ---- V_star ----
            vstar = {}
            for comp in (0, 1):
                for half in ("a", "b"):
                    vt = (va, vb)[half == "b"] if comp == 0 else (wa, wb)[half == "b"]
                    v0t = (va, vb)[half == "b"]
                    v1t = (wa, wb)[half == "b"]
                    pt = (pa, pb)[half == "b"]
                    ps_lin = pspool2.tile([TB, N], F32, tag="pslin")
                    nc.tensor.matmul(ps_lin[:], smt["lin"][:], vt[:, 1:N + 1],
                                     start=True, stop=False)
                    nc.tensor.matmul(ps_lin[:], smt["eyelapn"][:], vt[:, 0:N],
                                     start=False, stop=False)
                    last_lin = comp == 0
                    nc.tensor.matmul(ps_lin[:], smt["eyelapn"][:], vt[:, 2:N + 2],
                                     start=False, stop=False)
                    if comp == 0:
                        nc.tensor.matmul(ps_lin[:], smt["gpx"][:], pt[:, 1:N + 1],
                                         start=False, stop=True)
                    else:
                        nc.tensor.matmul(ps_lin[:], smt["eyegp"][:], pt[:, 2:N + 2],
                                         start=False, stop=False)
                        nc.tensor.matmul(ps_lin[:], smt["eyegpn"][:], pt[:, 0:N],
                                         start=False, stop=True)
                    ps_dx = pspool2.tile([TB, N], F32, tag="psdx")
                    nc.tensor.matmul(ps_dx[:], smt["gx"][:], vt[:, 1:N + 1],
                                     start=True, stop=True)
                    yd = spool.tile([TB, N], F32, tag="yd")
                    nc.vector.tensor_sub(yd[:], vt[:, 2:N + 2], vt[:, 0:N])
                    m2 = spool.tile([TB, N], F32, tag="m2")
                    nc.gpsimd.tensor_mul(m2[:], v1t[:, 1:N + 1], yd[:])
                    m1 = spool.tile([TB, N], F32, tag="m1")
                    nc.vector.tensor_mul(m1[:], v0t[:, 1:N + 1], ps_dx[:])
                    adv = spool.tile([TB, N], F32, tag="adv")
                    nc.vector.scalar_tensor_tensor(adv[:], m2[:], 0.5, m1[:],
                                                   AOP.mult, AOP.add)
                    vs = dpool.tile([TB, N], F32, tag=f"vs{comp}{half}")
                    nc.vector.scalar_tensor_tensor(vs[:], adv[:], CADV, ps_lin[:],
                                                   AOP.mult, AOP.add)
                    vstar[(comp, half)] = vs

            # ---- B' (owned + halo) with y-wrap padding ----
            bpo = dpool.tile([OWN, W], F32, tag="bpo")
            bph = dpool.tile([H2, W], F32, tag="bph")
            for half in ("a", "b"):
                ps_b = pspool2.tile([TB, N], F32, tag="psdx")
                nc.tensor.matmul(ps_b[:], smt["bdx"][:], vstar[(0, half)][:],
                                 start=True, stop=True)
                vs1 = vstar[(1, half)]
                ydb = spool.tile([TB, N], F32, tag="yd")
                nc.vector.tensor_sub(ydb[:, 1:N - 1], vs1[:, 2:N], vs1[:, 0:N - 2])
                nc.vector.tensor_sub(ydb[:, 0:1], vs1[:, 1:2], vs1[:, N - 1:N])
                nc.vector.tensor_sub(ydb[:, N - 1:N], vs1[:, 0:1],
                                     vs1[:, N - 2:N - 1])
                bp = spool.tile([TB, N], F32, tag=f"bp{half}")
                nc.vector.scalar_tensor_tensor(bp[:], ydb[:], CBD, ps_b[:],
                                               AOP.mult, AOP.add)
                if half == "a":
                    nc.sync.dma_start(bpo[0:68, K:K + N], bp[52:120, :])
                    nc.sync.dma_start(bph[0:K, K:K + N], bp[2:K + 2, :])
                else:
                    nc.sync.dma_start(bpo[68:OWN, K:K + N], bp[2:62, :])
                    nc.sync.dma_start(bph[K:H2, K:K + N], bp[62:112, :])
            nc.vector.tensor_copy(bpo[:, 0:K], bpo[:, N:N + K])
            nc.vector.tensor_copy(bpo[:, N + K:W], bpo[:, K:2 * K])
            nc.vector.tensor_copy(bph[:, 0:K], bph[:, N:N + K])
            nc.vector.tensor_copy(bph[:, N + K:W], bph[:, K:2 * K])

            # ---- B50 = Q(S) B', streaming Q stationaries from DRAM ----
            b50 = dpool.tile([OWN, N], F32, tag="b50")
            ps_q = pspool.tile([OWN, N], F32, tag="psph")
            CH = 8
            nq_m = 2 * K - 1
            first = True
            for base in range(0, nq_m, CH):
                cnt = min(CH, nq_m - base)
                qt_ = qpool.tile([OWN, CH * OWN], F32, tag="qm")
                nc.sync.dma_start(qt_[:, 0:cnt * OWN],
                                  dram_in["mqt"][:, base * OWN:(base + cnt) * OWN])
                for j in range(cnt):
                    dy = base + j - (K - 1)
                    nc.tensor.matmul(ps_q[:], qt_[:, j * OWN:(j + 1) * OWN],
                                     bpo[:, K + dy:K + dy + N],
                                     start=first, stop=False)
                    first = False
            nq_h = 2 * K - 3
            for base in range(0, nq_h, CH):
                cnt = min(CH, nq_h - base)
                qt_ = qpool.tile([H2, CH * OWN], F32, tag="qh")
                nc.sync.dma_start(qt_[:, 0:cnt * OWN],
                                  dram_in["hqt"][:, base * OWN:(base + cnt) * OWN])
                for j in range(cnt):
                    dy = base + j - (K - 2)
                    nc.tensor.matmul(ps_q[:], qt_[:, j * OWN:(j + 1) * OWN],
                                     bph[:, K + dy:K + dy + N],
                                     start=False, stop=(base + j == nq_h - 1))
            nc.scalar.copy(b50[:], ps_q[:])

            # ---- init phi from P ----
            phio = phipool.tile([OWN, W], F32, tag="phio")
            nc.sync.dma_start(phio[:, K:K + N], pblk[PADL:PADL + OWN, :])
            nc.sync.dma_start(phio[:, 0:K], pblk[PADL:PADL + OWN, N - K:N])
            nc.sync.dma_start(phio[:, N + K:W], pblk[PADL:PADL + OWN, 0:K])
            phih = phipool.tile([H2, W], F32, tag="phih")
            nc.sync.dma_start(phih[0:K, K:K + N], pblk[PADL - K:PADL, :])
            nc.sync.dma_start(phih[K:H2, K:K + N],
                              pblk[PADL + OWN:PADL + OWN + K, :])
            nc.sync.dma_start(phih[0:K, 0:K], pblk[PADL - K:PADL, N - K:N])
            nc.sync.dma_start(phih[0:K, N + K:W], pblk[PADL - K:PADL, 0:K])
            nc.sync.dma_start(phih[K:H2, 0:K],
                              pblk[PADL + OWN:PADL + OWN + K, N - K:N])
            nc.sync.dma_start(phih[K:H2, N + K:W],
                              pblk[PADL + OWN:PADL + OWN + K, 0:K])

            pid = nc.sync.partition_id()
            off_up = ((pid + 3) % 4) * H2 + K
            off_dn = ((pid + 1) % 4) * H2

            # ---- phases ----
            for ph in range(NPH):
                ps = pspool.tile([OWN, N], F32, tag="psph")
                for dyi in range(2 * K + 1):
                    dy = dyi - K
                    nc.tensor.matmul(ps[:], mst[:, dyi * OWN:(dyi + 1) * OWN],
                                     phio[:, K + dy:K + dy + N],
                                     start=(dyi == 0), stop=False)
                nc.tensor.matmul(ps[:], smt["negi"][:], b50[:],
                                 start=False, stop=False)
                for dyi in range(2 * K - 1):
                    dy = dyi - (K - 1)
                    nc.tensor.matmul(ps[:], hst[:, dyi * OWN:(dyi + 1) * OWN],
                                     phih[:, K + dy:K + dy + N],
                                     start=False, stop=(dyi == 2 * K - 2))
                phio = phipool.tile([OWN, W], F32, tag="phio")
                nc.scalar.copy(phio[:, K:K + N], ps[:])
                nc.vector.tensor_copy(phio[:, 0:K], phio[:, N:N + K])
                nc.vector.tensor_copy(phio[:, N + K:W], phio[:, K:2 * K])

                agi = drpool.tile([H2, N], F32, tag="agi")
                ago = drpool.tile([4 * H2, N], F32, tag="ago")
                nc.sync.dma_start(agi[0:K, :], phio[0:K, K:K + N])
                nc.sync.dma_start(agi[K:H2, :], phio[OWN - K:OWN, K:K + N])
                nc.gpsimd.collective_compute(
                    "AllGather", AOP.bypass, replica_groups=groups,
                    ins=[agi[:]], outs=[ago[:]])
                phih = phipool.tile([H2, W], F32, tag="phih")
                nc.sync.dma_start(phih[0:K, K:K + N], ago[ds(off_up, K), :])
                nc.sync.dma_start(phih[K:H2, K:K + N], ago[ds(off_dn, K), :])
                nc.vector.tensor_copy(phih[:, 0:K], phih[:, N:N + K])
                nc.vector.tensor_copy(phih[:, N + K:W], phih[:, K:2 * K])

            # ---- outputs ----
            ps0 = pspool2.tile([OWN, N], F32, tag="pslin")
            nc.tensor.matmul(ps0[:], smt["sela"][:], vstar[(0, "a")][:],
                             start=True, stop=False)
            nc.tensor.matmul(ps0[:], smt["selb"][:], vstar[(0, "b")][:],
                             start=False, stop=False)
            nc.tensor.matmul(ps0[:], smt["gphi"][:], phio[:, K:K + N],
                             start=False, stop=False)
            nc.tensor.matmul(ps0[:], smt["gphih"][:], phih[:, K:K + N],
                             start=False, stop=True)
            vn0 = spool.tile([OWN, N], F32, tag="vn0")
            nc.scalar.copy(vn0[:], ps0[:])
            nc.sync.dma_start(vout[0], vn0[:])

            ps1 = pspool2.tile([OWN, N], F32, tag="psdx")
            nc.tensor.matmul(ps1[:], smt["sela"][:], vstar[(1, "a")][:],
                             start=True, stop=False)
            nc.tensor.matmul(ps1[:], smt["selb"][:], vstar[(1, "b")][:],
                             start=False, stop=True)
            ydp = spool.tile([OWN, N], F32, tag="ydp")
            nc.vector.tensor_sub(ydp[:], phio[:, K + 1:K + 1 + N],
                                 phio[:, K - 1:K - 1 + N])
            vn1 = spool.tile([OWN, N], F32, tag="vn1")
            nc.vector.scalar_tensor_tensor(vn1[:], ydp[:], CGF, ps1[:],
                                           AOP.mult, AOP.add)
            nc.sync.dma_start(vout[1], vn1[:])

            pn = spool.tile([OWN, N], F32, tag="pn")
            nc.vector.scalar_tensor_tensor(pn[:], pown[:], BETA,
                                           phio[:, K:K + N], AOP.mult, AOP.add)
            nc.sync.dma_start(pout[:], pn[:])

    nc.finalize()
    return nc


def kernel(V, P):
    global _PROG
    V = np.ascontiguousarray(V, np.float32)
    P = np.ascontiguousarray(P, np.float32)
    if _PROG is None:
        consts = _build_consts()
        nc = _build_program(consts)
        _PROG = (nc, consts)
    nc, consts = _PROG
    in_maps = []
    for c in range(NCORE):
        ch, xb = c // 4, c % 4
        x0 = OWN * xb
        rows = np.arange(x0 - PADL, x0 + OWN + PADR) % N
        m = {"vblk": np.ascontiguousarray(V[:, ch][:, rows, :]),
             "pblk": np.ascontiguousarray(P[ch][rows, :])}
        m.update(consts)
        in_maps.append(m)
    trace = os.environ.get("NSK_TRACE", "") == "1"
    res = run_bass_kernel_spmd(nc, in_maps, core_ids=list(range(NCORE)),
                               trace=trace)
    if trace:
        print(f"HW exec time: {res.exec_time_ns} ns")
        if res.instructions_and_trace:
            print("trace:", res.instructions_and_trace[1])
    V_new = np.empty((2, C, N, N), np.float32)
    P_new = np.empty((C, N, N), np.float32)
    for c in range(NCORE):
        ch, xb = c // 4, c % 4
        x0 = OWN * xb
        V_new[:, ch, x0:x0 + OWN, :] = res.results[c]["vout"]
        P_new[ch, x0:x0 + OWN, :] = res.results[c]["pout"]
    return V_new, P_new


# revision 14
# speedup vs baseline: 3.1715x; 3.1715x over previous
"""NavierStokesSplittingEuler trn2 kernel, 8-core SPMD.

Sharding: x-axis 4-way per channel (core c: channel c//4, x-rows
[128*(c%4), 128*(c%4)+128)).  The 1000 Jacobi iterations run as 20
applications of the fused 50-step operator  phi <- S^50 phi - B50
(S = quarter neighbor-sum; B50 = sum_{j<50} S^j B' precomputed on
device).  Each application is a stack of PSUM-accumulated PE matmuls:
for each y-offset dy, the x-coupling is a banded stationary matrix and
the y-shift is a strided read of a wrap-padded SBUF buffer.  The 50-row
halo is refreshed once per phase with a per-channel AllGather; each
core selects its ring neighbors' rows with partition-id-driven dynamic
DMA offsets.
"""
import os
import sys

for _p in ("/opt/trn_rl_repo", "/root/.axon_site/_ro/trn_rl_repo"):
    if os.path.isdir(_p) and _p not in sys.path:
        sys.path.append(_p)

import numpy as np
import concourse.bass as bass
import concourse.tile as tile
from concourse import bacc, mybir
from concourse.bass import ds
from concourse.bass_utils import run_bass_kernel_spmd

F32 = mybir.dt.float32
N = 512
C = 2
NCORE = 8
K = 50           # fused steps per phase
NPH = int(os.environ.get("NSK_NPH", "20"))  # phases; K*NPH iterations
OWN = 128        # owned x-rows per core
H2 = 2 * K       # halo rows (50 up + 50 down)
TB = 122         # V_star work-tile rows (2 tiles, overlapping)
PADL = 52
PADR = 60
BOFF = 118       # B-tile row offset in the input block
BLK = OWN + PADL + PADR  # 240 input rows per core
W = N + 2 * K            # wrap-padded width

DT, BETA, RHO, NU = 0.1, 0.5, 1.0, 0.1
CADV = -DT
CLAPC = 1.0 - 4.0 * DT * NU
CLAPN = DT * NU
CGP = -DT * BETA / RHO / 2.0    # coeff on raw (P[+1]-P[-1]) diffs
CBD = RHO / (4.0 * DT) / 2.0    # B' = CBD*(xdiff+ydiff) = 1.25*...
CGF = -DT / RHO / 2.0           # V_new grad(Phi) coeff on raw diffs


def _taps(k):
    t = np.array([[1.0]])
    for _ in range(k):
        p = np.zeros((t.shape[0] + 2, t.shape[1] + 2))
        for dx, dy in ((-1, 0), (1, 0), (0, -1), (0, 1)):
            p[1 + dx:1 + dx + t.shape[0], 1 + dy:1 + dy + t.shape[1]] += 0.25 * t
        t = p
    return t


def _qtaps(k):
    acc = np.zeros((2 * k - 1, 2 * k - 1))
    cur = np.array([[1.0]])
    for j in range(k):
        r = (2 * k - 1 - cur.shape[0]) // 2
        acc[r:r + cur.shape[0], r:r + cur.shape[1]] += cur
        if j < k - 1:
            p = np.zeros((cur.shape[0] + 2, cur.shape[1] + 2))
            for dx, dy in ((-1, 0), (1, 0), (0, -1), (0, 1)):
                p[1 + dx:1 + dx + cur.shape[0], 1 + dy:1 + dy + cur.shape[1]] += 0.25 * cur
            cur = p
    return acc


def _band(nrows, ncols, entries):
    s = np.zeros((nrows, ncols), np.float32)
    for off, cf in entries.items():
        for m in range(ncols):
            kk = m + off
            if 0 <= kk < nrows:
                s[kk, m] += cf
    return s


def _stack_t(stack):
    """[nq, P, M] -> host layout [P, nq*M] so DMA is a plain 2D copy."""
    nq, p, m = stack.shape
    return np.ascontiguousarray(
        np.transpose(stack, (1, 0, 2)).reshape(p, nq * m).astype(np.float32))


def _sep_consts(taps, rank, rad):
    """SVD-separate taps -> (usto [OWN, rank*OWN], usth [H2, rank*OWN],
    vcstk [128, rank*4*N]) fp32 for the two-stage apply."""
    U, s, Vt = np.linalg.svd(taps)
    w = (U[:, :rank] * s[:rank]).T
    v = Vt[:rank]

    def wa(r, dd):
        return w[r, dd + rad] if abs(dd) <= rad else 0.0

    usto = np.zeros((OWN, rank * OWN), np.float32)
    for r in range(rank):
        for xp in range(OWN):
            for n in range(max(0, xp - rad), min(OWN, xp + rad + 1)):
                usto[xp, r * OWN + n] = wa(r, xp - n)
    usth = np.zeros((H2, rank * OWN), np.float32)
    for r in range(rank):
        for rh in range(H2):
            g = rh - K if rh < K else OWN + rh - K
            for n in range(OWN):
                val = wa(r, g - n)
                if val:
                    usth[rh, r * OWN + n] = val
    vcstk = np.zeros((128, rank * 4 * N), np.float32)
    for r in range(rank):
        for tp_ in range(4):
            for p in range(128):
                yp = 128 * tp_ + p
                for dd in range(-rad, rad + 1):
                    n = (yp - dd) % N
                    vcstk[p, (r * 4 + tp_) * N + n] += v[r, dd + rad]
    return usto, usth, vcstk


RSEP = 6    # rank of S^50 taps (apply err ~3e-7)
RQ = 10     # rank of Q taps (apply err ~7e-8)


def _build_consts():
    usto, usth, vcstk = _sep_consts(_taps(K), RSEP, K)
    uqo, uqh, vqstk = _sep_consts(_qtaps(K), RQ, K - 1)

    sml = {}
    sml["lin"] = _band(TB, TB, {0: CLAPC, 1: CLAPN, -1: CLAPN})
    sml["eyelapn"] = (CLAPN * np.eye(TB)).astype(np.float32)
    sml["gx"] = _band(TB, TB, {1: 0.5, -1: -0.5})
    sml["gpx"] = _band(TB, TB, {1: CGP, -1: -CGP})
    sml["eyegp"] = (CGP * np.eye(TB)).astype(np.float32)
    sml["eyegpn"] = (-CGP * np.eye(TB)).astype(np.float32)
    sml["bdx"] = _band(TB, TB, {1: CBD, -1: -CBD})
    sml["negi"] = (-np.eye(OWN)).astype(np.float32)
    sml["gphi"] = _band(OWN, OWN, {1: CGF, -1: -CGF})
    gph = np.zeros((H2, OWN), np.float32)
    gph[K - 1, 0] = -CGF
    gph[K, OWN - 1] = CGF
    sml["gphih"] = gph
    sela = np.zeros((TB, OWN), np.float32)
    for m in range(0, 69):
        sela[m + 52, m] = 1.0
    selb = np.zeros((TB, OWN), np.float32)
    for m in range(69, OWN):
        selb[m - 66, m] = 1.0
    sml["sela"] = sela
    sml["selb"] = selb

    consts = {"usto": usto, "usth": usth, "vcstk": vcstk,
              "uqo": uqo, "uqh": uqh, "vqstk": vqstk}
    consts.update(sml)
    return consts


_PROG = None


def _build_program(consts):
    nc = bacc.Bacc("TRN2", target_bir_lowering=False, debug=False,
                   enable_asserts=True, num_devices=NCORE)
    vblk = nc.declare_dram_parameter("vblk", [2, BLK, N], F32, isOutput=False)
    pblk = nc.declare_dram_parameter("pblk", [BLK, N], F32, isOutput=False)
    dram_in = {k: nc.declare_dram_parameter(k, list(v.shape), F32, isOutput=False)
               for k, v in consts.items()}
    vout = nc.declare_dram_parameter("vout", [2, OWN, N], F32, isOutput=True)
    pout = nc.declare_dram_parameter("pout", [OWN, N], F32, isOutput=True)

    groups = [[0, 1, 2, 3], [4, 5, 6, 7]]
    AOP = mybir.AluOpType
    SMALL = ("lin", "eyelapn", "gx", "gpx", "eyegp", "eyegpn", "bdx",
             "negi", "gphi", "gphih", "sela", "selb")

    with tile.TileContext(nc) as tc:
        with tc.tile_pool(name="const", bufs=1) as cpool, \
             tc.tile_pool(name="data", bufs=1) as dpool, \
             tc.tile_pool(name="phib", bufs=2) as phipool, \
             tc.tile_pool(name="qstream", bufs=2) as qpool, \
             tc.tile_pool(name="scratch", bufs=1) as spool, \
             tc.tile_pool(name="psph", bufs=2, space="PSUM") as pspool, \
             tc.tile_pool(name="psaux", bufs=2, space="PSUM") as pspool2, \
             tc.tile_pool(name="dram", bufs=2, space="DRAM") as drpool:

            # resident separable-apply constants
            usto = cpool.tile([OWN, RSEP * OWN], F32, tag="usto")
            nc.sync.dma_start(usto[:], dram_in["usto"][:])
            usth = cpool.tile([H2, RSEP * OWN], F32, tag="usth")
            nc.sync.dma_start(usth[:], dram_in["usth"][:])
            vcstk = cpool.tile([128, RSEP * 4 * N], F32, tag="vcstk")
            nc.sync.dma_start(vcstk[:], dram_in["vcstk"][:])
            uqo = cpool.tile([OWN, RQ * OWN], F32, tag="uqo")
            nc.sync.dma_start(uqo[:], dram_in["uqo"][:])
            uqh = cpool.tile([H2, RQ * OWN], F32, tag="uqh")
            nc.sync.dma_start(uqh[:], dram_in["uqh"][:])
            smt = {}
            for k in SMALL:
                smt[k] = cpool.tile(list(consts[k].shape), F32, tag=f"sm_{k}",
                                    name=f"sm_{k}")
                nc.sync.dma_start(smt[k][:], dram_in[k][:])

            # input field tiles: A = blk rows 0..117, B = rows 114..231
            va = dpool.tile([TB, N + 2], F32, tag="va0")
            vb = dpool.tile([TB, N + 2], F32, tag="vb0")
            wa = dpool.tile([TB, N + 2], F32, tag="wa1")
            wb = dpool.tile([TB, N + 2], F32, tag="wb1")
            pa = dpool.tile([TB, N + 2], F32, tag="pa")
            pb = dpool.tile([TB, N + 2], F32, tag="pb")
            for t, src, r0 in ((va, 0, 0), (vb, 0, BOFF), (wa, 1, 0), (wb, 1, BOFF)):
                nc.sync.dma_start(t[:, 1:N + 1], vblk[src, r0:r0 + TB, :])
                nc.sync.dma_start(t[:, 0:1], vblk[src, r0:r0 + TB, N - 1:N])
                nc.sync.dma_start(t[:, N + 1:N + 2], vblk[src, r0:r0 + TB, 0:1])
            for t, r0 in ((pa, 0), (pb, BOFF)):
                nc.sync.dma_start(t[:, 1:N + 1], pblk[r0:r0 + TB, :])
                nc.sync.dma_start(t[:, 0:1], pblk[r0:r0 + TB, N - 1:N])
                nc.sync.dma_start(t[:, N + 1:N + 2], pblk[r0:r0 + TB, 0:1])
            pown = dpool.tile([OWN, N], F32, tag="pown")
            nc.sync.dma_start(pown[:], pblk[PADL:PADL + OWN, :])

            # ---- V_star ----
            vstar = {}
            for comp in (0, 1):
                for half in ("a", "b"):
                    vt = (va, vb)[half == "b"] if comp == 0 else (wa, wb)[half == "b"]
                    v0t = (va, vb)[half == "b"]
                    v1t = (wa, wb)[half == "b"]
                    pt = (pa, pb)[half == "b"]
                    ps_lin = pspool2.tile([TB, N], F32, tag="pslin")
                    nc.tensor.matmul(ps_lin[:], smt["lin"][:], vt[:, 1:N + 1],
                                     start=True, stop=False)
                    nc.tensor.matmul(ps_lin[:], smt["eyelapn"][:], vt[:, 0:N],
                                     start=False, stop=False)
                    last_lin = comp == 0
                    nc.tensor.matmul(ps_lin[:], smt["eyelapn"][:], vt[:, 2:N + 2],
                                     start=False, stop=False)
                    if comp == 0:
                        nc.tensor.matmul(ps_lin[:], smt["gpx"][:], pt[:, 1:N + 1],
                                         start=False, stop=True)
                    else:
                        nc.tensor.matmul(ps_lin[:], smt["eyegp"][:], pt[:, 2:N + 2],
                                         start=False, stop=False)
                        nc.tensor.matmul(ps_lin[:], smt["eyegpn"][:], pt[:, 0:N],
                                         start=False, stop=True)
                    ps_dx = pspool2.tile([TB, N], F32, tag="psdx")
                    nc.tensor.matmul(ps_dx[:], smt["gx"][:], vt[:, 1:N + 1],
                                     start=True, stop=True)
                    yd = spool.tile([TB, N], F32, tag="yd")
                    nc.vector.tensor_sub(yd[:], vt[:, 2:N + 2], vt[:, 0:N])
                    m2 = spool.tile([TB, N], F32, tag="m2")
                    nc.gpsimd.tensor_mul(m2[:], v1t[:, 1:N + 1], yd[:])
                    m1 = spool.tile([TB, N], F32, tag="m1")
                    nc.vector.tensor_mul(m1[:], v0t[:, 1:N + 1], ps_dx[:])
                    adv = spool.tile([TB, N], F32, tag="adv")
                    nc.vector.scalar_tensor_tensor(adv[:], m2[:], 0.5, m1[:],
                                                   AOP.mult, AOP.add)
                    vs = dpool.tile([TB, N], F32, tag=f"vs{comp}{half}")
                    nc.vector.scalar_tensor_tensor(vs[:], adv[:], CADV, ps_lin[:],
                                                   AOP.mult, AOP.add)
                    vstar[(comp, half)] = vs

            # ---- B' (owned + halo) ----
            bpo = dpool.tile([OWN, N], F32, tag="bpo")
            bph = dpool.tile([H2, N], F32, tag="bph")
            for half in ("a", "b"):
                ps_b = pspool2.tile([TB, N], F32, tag="psdx")
                nc.tensor.matmul(ps_b[:], smt["bdx"][:], vstar[(0, half)][:],
                                 start=True, stop=True)
                vs1 = vstar[(1, half)]
                ydb = spool.tile([TB, N], F32, tag="yd")
                nc.vector.tensor_sub(ydb[:, 1:N - 1], vs1[:, 2:N], vs1[:, 0:N - 2])
                nc.vector.tensor_sub(ydb[:, 0:1], vs1[:, 1:2], vs1[:, N - 1:N])
                nc.vector.tensor_sub(ydb[:, N - 1:N], vs1[:, 0:1],
                                     vs1[:, N - 2:N - 1])
                bp = spool.tile([TB, N], F32, tag=f"bp{half}")
                nc.vector.scalar_tensor_tensor(bp[:], ydb[:], CBD, ps_b[:],
                                               AOP.mult, AOP.add)
                if half == "a":
                    nc.sync.dma_start(bpo[0:68, :], bp[52:120, :])
                    nc.sync.dma_start(bph[0:K, :], bp[2:K + 2, :])
                else:
                    nc.sync.dma_start(bpo[68:OWN, :], bp[2:62, :])
                    nc.sync.dma_start(bph[K:H2, :], bp[62:112, :])

            def sep_apply(src_o, src_h, uo, uh, vstk_sb, vstk_dr, rank, extra):
                """Two-stage separable apply; returns the result psum
                [OWN, N].  vstk_sb: SBUF moving constants (or None to
                stream per-slice from vstk_dr).  extra: list of
                (lhsT, rhs) matmuls accumulated at the end.  Owned
                matmuls are emitted before halo matmuls so the PE has
                work while the halo AllGather is still in flight."""
                ngrp = (rank * OWN + 511) // 512
                ats, psgs = [], []
                for t in range(4):
                    at = spool.tile([128, rank * OWN], F32, tag=f"at{rank}",
                                    name=f"at{rank}_{t}", bufs=8)
                    grp = []
                    for g in range(ngrp):
                        c0 = g * 512
                        c1 = min(rank * OWN, c0 + 512)
                        psg = pspool2.tile([128, c1 - c0], F32,
                                           tag=("pslin", "psdx")[g % 2],
                                           name=f"ps1_{t}_{g}")
                        nc.tensor.matmul(psg[:], src_o[:, 128 * t:128 * t + 128],
                                         uo[:, c0:c1], start=True, stop=False)
                        grp.append((psg, c0, c1))
                    ats.append(at)
                    psgs.append(grp)
                for t in range(4):
                    for psg, c0, c1 in psgs[t]:
                        nc.tensor.matmul(psg[:], src_h[:, 128 * t:128 * t + 128],
                                         uh[:, c0:c1], start=False, stop=True)
                        nc.scalar.copy(ats[t][:, c0:c1], psg[:])
                ps2 = pspool.tile([OWN, N], F32, tag="psph", name="ps2")
                nmm = rank * 4 + len(extra)
                i = 0
                for r in range(rank):
                    for tp_ in range(4):
                        if vstk_sb is not None:
                            mov = vstk_sb[:, (r * 4 + tp_) * N:(r * 4 + tp_ + 1) * N]
                        else:
                            mv = qpool.tile([128, N], F32, tag="qmov",
                                            name=f"qmov_{r}_{tp_}")
                            nc.sync.dma_start(
                                mv[:], vstk_dr[:, (r * 4 + tp_) * N:(r * 4 + tp_ + 1) * N])
                            mov = mv[:]
                        nc.tensor.matmul(ps2[:], ats[tp_][:, r * OWN:(r + 1) * OWN],
                                         mov, start=(i == 0), stop=(i == nmm - 1))
                        i += 1
                for lh, rh_ in extra:
                    nc.tensor.matmul(ps2[:], lh, rh_, start=False,
                                     stop=(i == nmm - 1))
                    i += 1
                return ps2

            # ---- B50 = Q(S) B' via separable apply (vq streamed) ----
            b50 = dpool.tile([OWN, N], F32, tag="b50")
            ps_q = sep_apply(bpo, bph, uqo, uqh, None, dram_in["vqstk"], RQ, [])
            nc.scalar.copy(b50[:], ps_q[:])

            # ---- init phi from P ----
            phio = phipool.tile([OWN, N], F32, tag="phio")
            nc.sync.dma_start(phio[:], pblk[PADL:PADL + OWN, :])
            phih = phipool.tile([H2, N], F32, tag="phih")
            nc.sync.dma_start(phih[0:K, :], pblk[PADL - K:PADL, :])
            nc.sync.dma_start(phih[K:H2, :], pblk[PADL + OWN:PADL + OWN + K, :])

            pid = nc.sync.partition_id()
            off_up = ((pid + 3) % 4) * H2 + K
            off_dn = ((pid + 1) % 4) * H2

            # ---- phases ----
            for ph in range(NPH):
                ps = sep_apply(phio, phih, usto, usth, vcstk, None, RSEP,
                               [(smt["negi"][:], b50[:])])
                phio = phipool.tile([OWN, N], F32, tag="phio")
                nc.scalar.copy(phio[:], ps[:])

                agi = drpool.tile([H2, N], F32, tag="agi")
                ago = drpool.tile([4 * H2, N], F32, tag="ago")
                nc.sync.dma_start(agi[0:K, :], phio[0:K, :])
                nc.sync.dma_start(agi[K:H2, :], phio[OWN - K:OWN, :])
                nc.gpsimd.collective_compute(
                    "AllGather", AOP.bypass, replica_groups=groups,
                    ins=[agi[:]], outs=[ago[:]])
                phih = phipool.tile([H2, N], F32, tag="phih")
                nc.sync.dma_start(phih[0:K, :], ago[ds(off_up, K), :])
                nc.sync.dma_start(phih[K:H2, :], ago[ds(off_dn, K), :])

            # ---- outputs ----
            ps0 = pspool2.tile([OWN, N], F32, tag="pslin")
            nc.tensor.matmul(ps0[:], smt["sela"][:], vstar[(0, "a")][:],
                             start=True, stop=False)
            nc.tensor.matmul(ps0[:], smt["selb"][:], vstar[(0, "b")][:],
                             start=False, stop=False)
            nc.tensor.matmul(ps0[:], smt["gphi"][:], phio[:],
                             start=False, stop=False)
            nc.tensor.matmul(ps0[:], smt["gphih"][:], phih[:],
                             start=False, stop=True)
            vn0 = spool.tile([OWN, N], F32, tag="vn0")
            nc.scalar.copy(vn0[:], ps0[:])
            nc.sync.dma_start(vout[0], vn0[:])

            ps1 = pspool2.tile([OWN, N], F32, tag="psdx")
            nc.tensor.matmul(ps1[:], smt["sela"][:], vstar[(1, "a")][:],
                             start=True, stop=False)
            nc.tensor.matmul(ps1[:], smt["selb"][:], vstar[(1, "b")][:],
                             start=False, stop=True)
            ydp = spool.tile([OWN, N], F32, tag="ydp")
            nc.vector.tensor_sub(ydp[:, 1:N - 1], phio[:, 2:N], phio[:, 0:N - 2])
            nc.vector.tensor_sub(ydp[:, 0:1], phio[:, 1:2], phio[:, N - 1:N])
            nc.vector.tensor_sub(ydp[:, N - 1:N], phio[:, 0:1],
                                 phio[:, N - 2:N - 1])
            vn1 = spool.tile([OWN, N], F32, tag="vn1")
            nc.vector.scalar_tensor_tensor(vn1[:], ydp[:], CGF, ps1[:],
                                           AOP.mult, AOP.add)
            nc.sync.dma_start(vout[1], vn1[:])

            pn = spool.tile([OWN, N], F32, tag="pn")
            nc.vector.scalar_tensor_tensor(pn[:], pown[:], BETA,
                                           phio[:], AOP.mult, AOP.add)
            nc.sync.dma_start(pout[:], pn[:])

    nc.finalize()
    return nc


def kernel(V, P):
    global _PROG
    V = np.ascontiguousarray(V, np.float32)
    P = np.ascontiguousarray(P, np.float32)
    if _PROG is None:
        consts = _build_consts()
        nc = _build_program(consts)
        _PROG = (nc, consts)
    nc, consts = _PROG
    in_maps = []
    for c in range(NCORE):
        ch, xb = c // 4, c % 4
        x0 = OWN * xb
        rows = np.arange(x0 - PADL, x0 + OWN + PADR) % N
        m = {"vblk": np.ascontiguousarray(V[:, ch][:, rows, :]),
             "pblk": np.ascontiguousarray(P[ch][rows, :])}
        m.update(consts)
        in_maps.append(m)
    trace = os.environ.get("NSK_TRACE", "") == "1"
    res = run_bass_kernel_spmd(nc, in_maps, core_ids=list(range(NCORE)),
                               trace=trace)
    if trace:
        print(f"HW exec time: {res.exec_time_ns} ns")
        if res.instructions_and_trace:
            print("trace:", res.instructions_and_trace[1])
    V_new = np.empty((2, C, N, N), np.float32)
    P_new = np.empty((C, N, N), np.float32)
    for c in range(NCORE):
        ch, xb = c // 4, c % 4
        x0 = OWN * xb
        V_new[:, ch, x0:x0 + OWN, :] = res.results[c]["vout"]
        P_new[ch, x0:x0 + OWN, :] = res.results[c]["pout"]
    return V_new, P_new


# revision 15
# speedup vs baseline: 17.0160x; 5.3653x over previous
"""NavierStokesSplittingEuler trn2 kernel, 8-core SPMD — single-shot
folded-operator design.

Sharding: x-axis 4-way per channel (core c: channel c//4, x-rows
[128*(c%4), 128*(c%4)+128)).  The 1000 Jacobi iterations are evaluated
in CLOSED FORM: phi_1000 = C_S * P - C_Q * B'  where C_S = IDFT(s^1000)
and C_Q = IDFT((1-s^1000)/(1-s)) are 512-periodic circulant kernels
(s = Jacobi symbol), both strongly separable (SVD rank 4 and 12).
Each apply is two stages of PE matmuls: stage 1 contracts the x-axis
against per-rank circulant profiles with the field as the stationary
operand (fusing the transpose); stage 2 contracts y against circulant
profile slices, accumulating all ranks into one PSUM tile (fusing the
transpose back).  Cross-core communication is two AllGathers total:
one to share B' (overlapped with the S-apply) and a 2-row one for the
final pressure gradient.
"""
import os
import sys

for _p in ("/opt/trn_rl_repo", "/root/.axon_site/_ro/trn_rl_repo"):
    if os.path.isdir(_p) and _p not in sys.path:
        sys.path.append(_p)

import numpy as np
import concourse.bass as bass
import concourse.tile as tile
from concourse import bacc, mybir
from concourse.bass import ds
from concourse.bass_utils import run_bass_kernel_spmd

F32 = mybir.dt.float32
N = 512
C = 2
NCORE = 8
NIT = 50 * int(os.environ.get("NSK_NPH", "20"))  # jacobi iterations
OWN = 128
TB = 122         # V_star work-tile rows (A/B tiles)
PADL = 52
PADR = 60
BOFF = 118       # B-tile row offset in the input block
BLK = OWN + PADL + PADR  # 240 input rows per core
RS1 = 4          # rank of folded S^NIT kernel
RQ1 = 12         # rank of folded Q kernel

DT, BETA, RHO, NU = 0.1, 0.5, 1.0, 0.1
CADV = -DT
CLAPC = 1.0 - 4.0 * DT * NU
CLAPN = DT * NU
CGP = -DT * BETA / RHO / 2.0    # coeff on raw (P[+1]-P[-1]) diffs
CBD = RHO / (4.0 * DT) / 2.0    # B' = CBD*(xdiff+ydiff)
CGF = -DT / RHO / 2.0           # V_new grad(Phi) coeff on raw diffs


def _band(nrows, ncols, entries):
    s = np.zeros((nrows, ncols), np.float32)
    for off, cf in entries.items():
        for m in range(ncols):
            kk = m + off
            if 0 <= kk < nrows:
                s[kk, m] += cf
    return s


def _fold_kernels(nit):
    kx = np.arange(N)
    c1 = np.cos(2 * np.pi * kx / N)
    s = (c1[:, None] + c1[None, :]) / 2.0
    sn = s**nit
    with np.errstate(divide='ignore', invalid='ignore'):
        q = np.where(np.abs(1 - s) < 1e-14, float(nit), (1 - sn) / (1 - s))
    CS = np.real(np.fft.ifft2(sn))
    CQ = np.real(np.fft.ifft2(q))
    return CS, CQ


def _sep_profiles(Ck, rank, neg=False):
    U, sv, Vt = np.linalg.svd(Ck)
    u = (U[:, :rank] * sv[:rank]).T
    v = Vt[:rank]
    if neg:
        v = -v
    return u, v


def _build_usf(u, rank):
    """Stage-1 moving constants, rotated frame: [128, 4*rank*128];
    slice xt: usf[:, xt*rank*128 : (xt+1)*rank*128]."""
    idx_p = np.arange(128)[:, None]
    idx_n = np.arange(128)[None, :]
    out = np.zeros((128, 4 * rank * 128), np.float32)
    for xt in range(4):
        for r in range(rank):
            blk = u[r][(128 * xt + idx_p - idx_n) % N]
            out[:, xt * rank * 128 + r * 128:
                xt * rank * 128 + (r + 1) * 128] = blk
    return out


def _build_vc(v, rank):
    """Stage-2 moving constants [128, rank*4*N]."""
    out = np.zeros((128, rank * 4 * N), np.float32)
    idx_p = np.arange(128)[:, None]
    idx_n = np.arange(N)[None, :]
    for r in range(rank):
        for tp in range(4):
            out[:, (r * 4 + tp) * N:(r * 4 + tp + 1) * N] = \
                v[r][(128 * tp + idx_p - idx_n) % N]
    return out


def _build_consts():
    CS, CQ = _fold_kernels(NIT)
    uS, vS = _sep_profiles(CS, RS1)
    uQ, vQ = _sep_profiles(CQ, RQ1, neg=True)

    sml = {}
    sml["lin"] = _band(TB, TB, {0: CLAPC, 1: CLAPN, -1: CLAPN})
    sml["eyelapn"] = (CLAPN * np.eye(TB)).astype(np.float32)
    sml["gx"] = _band(TB, TB, {1: 0.5, -1: -0.5})
    sml["gpx"] = _band(TB, TB, {1: CGP, -1: -CGP})
    sml["eyegp"] = (CGP * np.eye(TB)).astype(np.float32)
    sml["eyegpn"] = (-CGP * np.eye(TB)).astype(np.float32)
    sml["bdx"] = _band(TB, TB, {1: CBD, -1: -CBD})
    sml["gphi"] = _band(OWN, OWN, {1: CGF, -1: -CGF})
    gph2 = np.zeros((2, OWN), np.float32)
    gph2[0, 0] = -CGF        # phi[x0-1] term at out row 0
    gph2[1, OWN - 1] = CGF   # phi[x0+128] term at out row 127
    sml["gphih"] = gph2
    sela = np.zeros((TB, OWN), np.float32)
    for m in range(0, 69):
        sela[m + 52, m] = 1.0
    selb = np.zeros((TB, OWN), np.float32)
    for m in range(69, OWN):
        selb[m - 66, m] = 1.0
    sml["sela"] = sela
    sml["selb"] = selb

    consts = {"usfs": _build_usf(uS, RS1), "vcs": _build_vc(vS, RS1),
              "usfq": _build_usf(uQ, RQ1), "vcq": _build_vc(vQ, RQ1)}
    consts.update(sml)
    return consts


_PROG = None


def _build_program(consts):
    nc = bacc.Bacc("TRN2", target_bir_lowering=False, debug=False,
                   enable_asserts=True, num_devices=NCORE)
    vblk = nc.declare_dram_parameter("vblk", [2, BLK, N], F32, isOutput=False)
    pblk = nc.declare_dram_parameter("pblk", [BLK, N], F32, isOutput=False)
    pfull = nc.declare_dram_parameter("pfull", [N, N], F32, isOutput=False)
    dram_in = {k: nc.declare_dram_parameter(k, list(v.shape), F32,
                                            isOutput=False)
               for k, v in consts.items()}
    vout = nc.declare_dram_parameter("vout", [2, OWN, N], F32, isOutput=True)
    pout = nc.declare_dram_parameter("pout", [OWN, N], F32, isOutput=True)

    groups = [[0, 1, 2, 3], [4, 5, 6, 7]]
    AOP = mybir.AluOpType
    SMALL = ("lin", "eyelapn", "gx", "gpx", "eyegp", "eyegpn", "bdx",
             "gphi", "gphih", "sela", "selb")
    QW = RQ1 * 128   # 1536 = 3 groups of 512

    with tile.TileContext(nc) as tc:
        with tc.tile_pool(name="const", bufs=1) as cpool, \
             tc.tile_pool(name="data", bufs=1) as dpool, \
             tc.tile_pool(name="qstream", bufs=6) as qpool, \
             tc.tile_pool(name="scratch", bufs=1) as spool, \
             tc.tile_pool(name="psA", bufs=2, space="PSUM") as psa, \
             tc.tile_pool(name="psB", bufs=2, space="PSUM") as psb, \
             tc.tile_pool(name="dram", bufs=1, space="DRAM") as drpool:

            # ---- constants ----
            usfs = cpool.tile([128, 4 * RS1 * 128], F32, tag="usfs")
            nc.sync.dma_start(usfs[:], dram_in["usfs"][:])
            vcs = cpool.tile([128, RS1 * 4 * N], F32, tag="vcs")
            nc.sync.dma_start(vcs[:], dram_in["vcs"][:])
            usfq = cpool.tile([128, 4 * QW], F32, tag="usfq")
            nc.sync.dma_start(usfq[:], dram_in["usfq"][:])
            smt = {}
            for k in SMALL:
                smt[k] = cpool.tile(list(consts[k].shape), F32, tag=f"sm_{k}",
                                    name=f"sm_{k}")
                nc.sync.dma_start(smt[k][:], dram_in[k][:])

            # ---- field inputs ----
            va = dpool.tile([TB, N + 2], F32, tag="va0")
            vb = dpool.tile([TB, N + 2], F32, tag="vb0")
            wa = dpool.tile([TB, N + 2], F32, tag="wa1")
            wb = dpool.tile([TB, N + 2], F32, tag="wb1")
            pa = dpool.tile([TB, N + 2], F32, tag="pa")
            pb = dpool.tile([TB, N + 2], F32, tag="pb")
            for t, src, r0 in ((va, 0, 0), (vb, 0, BOFF), (wa, 1, 0),
                               (wb, 1, BOFF)):
                nc.sync.dma_start(t[:, 1:N + 1], vblk[src, r0:r0 + TB, :])
                nc.sync.dma_start(t[:, 0:1], vblk[src, r0:r0 + TB, N - 1:N])
                nc.sync.dma_start(t[:, N + 1:N + 2], vblk[src, r0:r0 + TB, 0:1])
            for t, r0 in ((pa, 0), (pb, BOFF)):
                nc.sync.dma_start(t[:, 1:N + 1], pblk[r0:r0 + TB, :])
                nc.sync.dma_start(t[:, 0:1], pblk[r0:r0 + TB, N - 1:N])
                nc.sync.dma_start(t[:, N + 1:N + 2], pblk[r0:r0 + TB, 0:1])
            pown = dpool.tile([OWN, N], F32, tag="pown")
            nc.sync.dma_start(pown[:], pblk[PADL:PADL + OWN, :])
            pf = []
            for xt in range(4):
                t = dpool.tile([128, N], F32, tag=f"pf{xt}", name=f"pf{xt}")
                nc.sync.dma_start(t[:], pfull[128 * xt:128 * xt + 128, :])
                pf.append(t)

            # ---- V_star ----
            vstar = {}
            for comp in (0, 1):
                for half in ("a", "b"):
                    vt = (va, vb)[half == "b"] if comp == 0 else (wa, wb)[half == "b"]
                    v0t = (va, vb)[half == "b"]
                    v1t = (wa, wb)[half == "b"]
                    pt = (pa, pb)[half == "b"]
                    ps_lin = psa.tile([TB, N], F32, tag="pslin")
                    nc.tensor.matmul(ps_lin[:], smt["lin"][:], vt[:, 1:N + 1],
                                     start=True, stop=False)
                    nc.tensor.matmul(ps_lin[:], smt["eyelapn"][:], vt[:, 0:N],
                                     start=False, stop=False)
                    nc.tensor.matmul(ps_lin[:], smt["eyelapn"][:], vt[:, 2:N + 2],
                                     start=False, stop=False)
                    if comp == 0:
                        nc.tensor.matmul(ps_lin[:], smt["gpx"][:], pt[:, 1:N + 1],
                                         start=False, stop=True)
                    else:
                        nc.tensor.matmul(ps_lin[:], smt["eyegp"][:], pt[:, 2:N + 2],
                                         start=False, stop=False)
                        nc.tensor.matmul(ps_lin[:], smt["eyegpn"][:], pt[:, 0:N],
                                         start=False, stop=True)
                    ps_dx = psb.tile([TB, N], F32, tag="psdx")
                    nc.tensor.matmul(ps_dx[:], smt["gx"][:], vt[:, 1:N + 1],
                                     start=True, stop=True)
                    yd = spool.tile([TB, N], F32, tag="yd")
                    nc.vector.tensor_sub(yd[:], vt[:, 2:N + 2], vt[:, 0:N])
                    m2 = spool.tile([TB, N], F32, tag="m2")
                    nc.gpsimd.tensor_mul(m2[:], v1t[:, 1:N + 1], yd[:])
                    m1 = spool.tile([TB, N], F32, tag="m1")
                    nc.vector.tensor_mul(m1[:], v0t[:, 1:N + 1], ps_dx[:])
                    adv = spool.tile([TB, N], F32, tag="adv")
                    nc.vector.scalar_tensor_tensor(adv[:], m2[:], 0.5, m1[:],
                                                   AOP.mult, AOP.add)
                    vs = dpool.tile([TB, N], F32, tag=f"vs{comp}{half}")
                    nc.vector.scalar_tensor_tensor(vs[:], adv[:], CADV, ps_lin[:],
                                                   AOP.mult, AOP.add)
                    vstar[(comp, half)] = vs

            # ---- B' on owned rows ----
            bpo = dpool.tile([OWN, N], F32, tag="bpo")
            for half in ("a", "b"):
                ps_b = psb.tile([TB, N], F32, tag="psdx")
                nc.tensor.matmul(ps_b[:], smt["bdx"][:], vstar[(0, half)][:],
                                 start=True, stop=True)
                vs1 = vstar[(1, half)]
                ydb = spool.tile([TB, N], F32, tag="yd")
                nc.vector.tensor_sub(ydb[:, 1:N - 1], vs1[:, 2:N], vs1[:, 0:N - 2])
                nc.vector.tensor_sub(ydb[:, 0:1], vs1[:, 1:2], vs1[:, N - 1:N])
                nc.vector.tensor_sub(ydb[:, N - 1:N], vs1[:, 0:1],
                                     vs1[:, N - 2:N - 1])
                bp = spool.tile([TB, N], F32, tag=f"bp{half}")
                nc.vector.scalar_tensor_tensor(bp[:], ydb[:], CBD, ps_b[:],
                                               AOP.mult, AOP.add)
                if half == "a":
                    nc.sync.dma_start(bpo[0:68, :], bp[52:120, :])
                else:
                    nc.sync.dma_start(bpo[68:OWN, :], bp[2:62, :])

            # ---- share B' across the x-ring (overlapped with S-apply) ----
            agB = drpool.tile([OWN, N], F32, tag="agB")
            agoB = drpool.tile([N, N], F32, tag="agoB")
            nc.sync.dma_start(agB[:], bpo[:])
            nc.gpsimd.collective_compute(
                "AllGather", AOP.bypass, replica_groups=groups,
                ins=[agB[:]], outs=[agoB[:]])
            pid = nc.sync.partition_id()
            bf = []
            for xt in range(4):
                t = dpool.tile([128, N], F32, tag=f"bf{xt}", name=f"bf{xt}")
                off = ((pid + xt) % 4) * 128
                nc.sync.dma_start(t[:], agoB[ds(off, 128), :])
                bf.append(t)

            psF = psa.tile([OWN, N], F32, tag="psF", name="psF")
            nmm_f = RS1 * 4 + RQ1 * 4
            imm = 0

            # ---- S-apply on P (no cross-core dependency) ----
            atS = []
            for tp in range(4):
                psg = psb.tile([128, RS1 * 128], F32, tag="ps1",
                               name=f"psgS{tp}")
                for xt in range(4):
                    nc.tensor.matmul(psg[:], pf[xt][:, 128 * tp:128 * tp + 128],
                                     usfs[:, xt * RS1 * 128:(xt + 1) * RS1 * 128],
                                     start=(xt == 0), stop=(xt == 3))
                at = spool.tile([128, RS1 * 128], F32, tag="atS",
                                name=f"atS{tp}", bufs=4)
                nc.scalar.copy(at[:], psg[:])
                atS.append(at)
            for r in range(RS1):
                for tp in range(4):
                    nc.tensor.matmul(psF[:], atS[tp][:, r * 128:(r + 1) * 128],
                                     vcs[:, (r * 4 + tp) * N:(r * 4 + tp + 1) * N],
                                     start=(imm == 0), stop=(imm == nmm_f - 1))
                    imm += 1

            # ---- Q-apply on B' (negated; waits on the AllGather) ----
            atQ = []
            for tp in range(4):
                at = spool.tile([128, QW], F32, tag="atQ", name=f"atQ{tp}",
                                bufs=4)
                for g in range(3):
                    c0 = g * 512
                    psg = psb.tile([128, 512], F32, tag="ps1",
                                   name=f"psgQ{tp}_{g}")
                    for xt in range(4):
                        nc.tensor.matmul(
                            psg[:], bf[xt][:, 128 * tp:128 * tp + 128],
                            usfq[:, xt * QW + c0:xt * QW + c0 + 512],
                            start=(xt == 0), stop=(xt == 3))
                    nc.scalar.copy(at[:, c0:c0 + 512], psg[:])
                atQ.append(at)
            for r in range(RQ1):
                for tp in range(4):
                    mv = qpool.tile([128, N], F32, tag="qmov",
                                    name=f"qmov{r}_{tp}")
                    nc.sync.dma_start(
                        mv[:],
                        dram_in["vcq"][:, (r * 4 + tp) * N:(r * 4 + tp + 1) * N])
                    nc.tensor.matmul(psF[:], atQ[tp][:, r * 128:(r + 1) * 128],
                                     mv[:], start=(imm == 0),
                                     stop=(imm == nmm_f - 1))
                    imm += 1

            phiow = dpool.tile([OWN, N], F32, tag="phiow")
            nc.scalar.copy(phiow[:], psF[:])

            # ---- 2-row boundary share for grad(Phi) ----
            agi2 = drpool.tile([2, N], F32, tag="agi2")
            ago2 = drpool.tile([8, N], F32, tag="ago2")
            nc.sync.dma_start(agi2[0:1, :], phiow[0:1, :])
            nc.sync.dma_start(agi2[1:2, :], phiow[OWN - 1:OWN, :])
            nc.gpsimd.collective_compute(
                "AllGather", AOP.bypass, replica_groups=groups,
                ins=[agi2[:]], outs=[ago2[:]])
            phih2 = dpool.tile([2, N], F32, tag="phih2")
            off_up = ((pid + 3) % 4) * 2 + 1
            off_dn = ((pid + 1) % 4) * 2
            nc.sync.dma_start(phih2[0:1, :], ago2[ds(off_up, 1), :])
            nc.sync.dma_start(phih2[1:2, :], ago2[ds(off_dn, 1), :])

            # ---- outputs ----
            ps0 = psa.tile([OWN, N], F32, tag="pslin")
            nc.tensor.matmul(ps0[:], smt["sela"][:], vstar[(0, "a")][:],
                             start=True, stop=False)
            nc.tensor.matmul(ps0[:], smt["selb"][:], vstar[(0, "b")][:],
                             start=False, stop=False)
            nc.tensor.matmul(ps0[:], smt["gphi"][:], phiow[:],
                             start=False, stop=False)
            nc.tensor.matmul(ps0[:], smt["gphih"][:], phih2[:],
                             start=False, stop=True)
            vn0 = spool.tile([OWN, N], F32, tag="vn0")
            nc.scalar.copy(vn0[:], ps0[:])
            nc.sync.dma_start(vout[0], vn0[:])

            ps1 = psb.tile([OWN, N], F32, tag="psdx")
            nc.tensor.matmul(ps1[:], smt["sela"][:], vstar[(1, "a")][:],
                             start=True, stop=False)
            nc.tensor.matmul(ps1[:], smt["selb"][:], vstar[(1, "b")][:],
                             start=False, stop=True)
            ydp = spool.tile([OWN, N], F32, tag="ydp")
            nc.vector.tensor_sub(ydp[:, 1:N - 1], phiow[:, 2:N],
                                 phiow[:, 0:N - 2])
            nc.vector.tensor_sub(ydp[:, 0:1], phiow[:, 1:2], phiow[:, N - 1:N])
            nc.vector.tensor_sub(ydp[:, N - 1:N], phiow[:, 0:1],
                                 phiow[:, N - 2:N - 1])
            vn1 = spool.tile([OWN, N], F32, tag="vn1")
            nc.vector.scalar_tensor_tensor(vn1[:], ydp[:], CGF, ps1[:],
                                           AOP.mult, AOP.add)
            nc.sync.dma_start(vout[1], vn1[:])

            pn = spool.tile([OWN, N], F32, tag="pn")
            nc.vector.scalar_tensor_tensor(pn[:], pown[:], BETA, phiow[:],
                                           AOP.mult, AOP.add)
            nc.sync.dma_start(pout[:], pn[:])

    nc.finalize()
    return nc


def kernel(V, P):
    global _PROG
    V = np.ascontiguousarray(V, np.float32)
    P = np.ascontiguousarray(P, np.float32)
    if _PROG is None:
        consts = _build_consts()
        nc = _build_program(consts)
        _PROG = (nc, consts)
    nc, consts = _PROG
    in_maps = []
    for c in range(NCORE):
        ch, xb = c // 4, c % 4
        x0 = OWN * xb
        rows = np.arange(x0 - PADL, x0 + OWN + PADR) % N
        m = {"vblk": np.ascontiguousarray(V[:, ch][:, rows, :]),
             "pblk": np.ascontiguousarray(P[ch][rows, :]),
             "pfull": np.ascontiguousarray(np.roll(P[ch], -x0, axis=0))}
        m.update(consts)
        in_maps.append(m)
    trace = os.environ.get("NSK_TRACE", "") == "1"
    res = run_bass_kernel_spmd(nc, in_maps, core_ids=list(range(NCORE)),
                               trace=trace)
    if trace:
        print(f"HW exec time: {res.exec_time_ns} ns")
        if res.instructions_and_trace:
            print("trace:", res.instructions_and_trace[1])
    V_new = np.empty((2, C, N, N), np.float32)
    P_new = np.empty((C, N, N), np.float32)
    for c in range(NCORE):
        ch, xb = c // 4, c % 4
        x0 = OWN * xb
        V_new[:, ch, x0:x0 + OWN, :] = res.results[c]["vout"]
        P_new[ch, x0:x0 + OWN, :] = res.results[c]["pout"]
    return V_new, P_new
